# revision 1
# baseline (speedup 1.0000x reference)
"""Trainium2 Bass kernel for LocalSLC GNN message passing.

Computation (per batch b):
    y[b,n,o] = sum_{k,i} bs[n,k] * ws[k,i,o] * x[b, knn_ids[n,k], i]

Shapes: B=16, N=10000, K=16, C_IN=C_OUT=64, fp32.

Strategy (8 NeuronCores, data-parallel over batch, 2 batches/core):
  * Host packs x for core c as xpair[n, 0:64]=x[2c], xpair[n, 64:128]=x[2c+1],
    so one gathered 512B row serves both batches (halves gather traffic and
    hits the >=512B/descriptor DMA sweet spot).
  * Per 128-node tile: one multi-index indirect DMA gathers the 16 neighbor
    rows per node from DRAM into SBUF G[128, 16, 128].
  * DVE tensor_scalar (per-partition scalars = bs tile columns, 2x_2P mode)
    scales G by bs in place.
  * Per k: PE transposes G[:, k, :] tiles into [(2b,i), n] chunks (PSUM,
    one accumulation group per bank), ACT escapes PSUM -> SBUF rounding to
    float32r, then a full-rate f32r PE matmul with a stationary
    block-diagonal W2[k] = diag(ws[k], ws[k]) accumulates y[(2b,o), n]
    over the 16 k's in one PSUM bank.
  * y escapes via DVE and DMAs out as yT[b, o, n]; host transposes back.
"""

import numpy as np

import concourse.bass as bass
import concourse.tile as tile
from concourse import bacc, mybir
from concourse.masks import make_identity

B, N, K, CI, CO = 16, 10000, 16, 64, 64
NCORES = 8
BPC = B // NCORES  # 2 batches per core
NPAD = 10240  # pad N to a multiple of 512
TS = 128  # nodes per tile


def round_f32r(a):
    """Round fp32 array to the float32r grid (11-bit mantissa, RNE)."""
    u = a.astype(np.float32).view(np.uint32)
    low = u & 0xFFF
    add = (low > 0x800) | ((low == 0x800) & (((u >> 12) & 1) == 1))
    return (((u >> 12) + add.astype(np.uint32)) << 12).view(np.float32)


def build_program(npad=NPAD, sb_tiles=4):
    """Build the per-core Bass program (identical on all 8 cores)."""
    nt = npad // TS
    nsb = nt // sb_tiles
    assert nsb * sb_tiles == nt
    sbn = sb_tiles * TS  # nodes per superblock (one psum bank: <=512 fp32)
    assert sbn <= 512

    nc = bacc.Bacc("TRN2", target_bir_lowering=False, debug=False)
    f32, f32r, i32 = mybir.dt.float32, mybir.dt.float32r, mybir.dt.int32

    nt_ = npad // TS
    xpair = nc.dram_tensor("xpair", [npad, 2 * CI], f32, kind="ExternalInput").ap()
    # wrapped int16 gather indices: per (tile, k-half) a [128, 64] block in
    # dma_gather's "wrapped in 16 partitions, replicated across cores" layout
    idsw = nc.dram_tensor(
        "idsw", [nt_, 2, 128, 64], mybir.dt.int16, kind="ExternalInput"
    ).ap()
    bsd = nc.dram_tensor("bs", [npad, K], f32, kind="ExternalInput").ap()
    # block-diag W2[k] = [[ws[k], 0], [0, ws[k]]], host-rounded to f32r
    wts = nc.dram_tensor("w2", [K, 2 * CI, 2 * CO], f32r, kind="ExternalInput").ap()
    yT = nc.dram_tensor("yT", [BPC, CO, npad], f32, kind="ExternalOutput").ap()

    with tile.TileContext(nc) as tc:
        with (
            tc.tile_pool(name="const", bufs=1) as const_pool,
            tc.tile_pool(name="meta", bufs=2 * sb_tiles) as meta_pool,
            tc.tile_pool(name="g", bufs=2 * sb_tiles) as g_pool,
            tc.tile_pool(name="zts", bufs=3) as zts_pool,
            tc.tile_pool(name="ysb", bufs=2) as ysb_pool,
            tc.tile_pool(name="ztp", bufs=3, space="PSUM") as ztp_pool,
            tc.tile_pool(name="yp", bufs=2, space="PSUM") as yp_pool,
        ):
            ident = const_pool.tile([128, 128], f32)
            make_identity(nc, ident[:])
            w2_s = const_pool.tile([128, K, 2 * CO], f32r)
            for k in range(K):
                nc.sync.dma_start(out=w2_s[:, k, :], in_=wts[k])

            for sbi in range(nsb):
                t0 = sbi * sb_tiles
                # one batched DMA per superblock for indices and bs scalars
                ids_sb = meta_pool.tile(
                    [128, sb_tiles, 2, 64], mybir.dt.int16, tag="ids"
                )
                nc.sync.dma_start(
                    out=ids_sb[:],
                    in_=idsw[t0:t0 + sb_tiles].rearrange("t h p s -> p t h s"),
                )
                bs_sb = meta_pool.tile([TS, sb_tiles, K], f32, tag="bs")
                nc.sync.dma_start(
                    out=bs_sb[:],
                    in_=bsd[t0 * TS:(t0 + sb_tiles) * TS, :].rearrange(
                        "(t p) k -> p t k", p=TS
                    ),
                )
                g_tiles = []
                for t in range(sb_tiles):
                    g = g_pool.tile([TS, K, 2 * CI], f32, tag="g")
                    for h in range(2):
                        nc.gpsimd.dma_gather(
                            out_ap=g[:, h * (K // 2):(h + 1) * (K // 2), :],
                            in_ap=xpair[:],
                            idxs_ap=ids_sb[:, t, h, :],
                            num_idxs=1024,
                            num_idxs_reg=1024,
                            elem_size=2 * CI,
                        )
                    for k in range(K):
                        nc.vector.tensor_scalar_mul(
                            g[:, k, :], g[:, k, :], bs_sb[:, t, k:k + 1]
                        )
                    g_tiles.append(g)

                y_ps = yp_pool.tile([2 * CO, sbn], f32, tag="y")
                for k in range(K):
                    zt_ps = ztp_pool.tile([128, sbn], f32, tag="ztp")
                    for t in range(sb_tiles):
                        nc.tensor.matmul(
                            zt_ps[:, t * TS:(t + 1) * TS],
                            lhsT=g_tiles[t][:, k, :],
                            rhs=ident[:],
                            is_transpose=True,
                            start=(t == 0),
                            stop=(t == sb_tiles - 1),
                        )
                    zt_sb = zts_pool.tile([128, sbn], f32r, tag="zts")
                    nc.scalar.copy(out=zt_sb[:], in_=zt_ps[:])
                    nc.tensor.matmul(
                        y_ps[:],
                        lhsT=w2_s[:, k, :],
                        rhs=zt_sb[:],
                        start=(k == 0),
                        stop=(k == K - 1),
                    )
                y_sb = ysb_pool.tile([2 * CO, sbn], f32, tag="ysb")
                nc.vector.tensor_copy(out=y_sb[:], in_=y_ps[:])
                for b in range(BPC):
                    nc.sync.dma_start(
                        out=yT[b, :, sbi * sbn:(sbi + 1) * sbn],
                        in_=y_sb[b * CO:(b + 1) * CO, :],
                    )

    nc.compile()
    return nc


_CACHE = {}


def _get_program():
    if "nc" not in _CACHE:
        _CACHE["nc"] = build_program()
    return _CACHE["nc"]


def _wrap_ids(ids_p, npad=NPAD):
    """Build dma_gather wrapped-int16 index blocks [nt, 2, 128, 64].

    Per (tile, k-half): flat order j = k_local*128 + n_local (so gathered
    row j lands on partition j%128, free slot j//128 = k_local), then
    wrapped w[p, s] = flat[s*16 + p] and replicated across the 8 Q7 cores.
    """
    nt = npad // TS
    a = ids_p.reshape(nt, TS, 2, K // 2)      # [t, n, h, kl]
    a = a.transpose(0, 2, 3, 1)               # [t, h, kl, n] -> flat kl*128+n
    f = a.reshape(nt, 2, 1024)
    w = f.reshape(nt, 2, 64, 16).transpose(0, 1, 3, 2)  # [t, h, 16, 64]
    return np.ascontiguousarray(np.tile(w, (1, 1, 8, 1)).astype(np.int16))


def _pack_inputs(x, knn_ids, bs, ws):
    """Host-side packing into per-core input maps."""
    ids_p = np.zeros((NPAD, K), np.int32)
    ids_p[:N] = knn_ids
    idsw = _wrap_ids(ids_p)
    bs_p = np.zeros((NPAD, K), np.float32)
    bs_p[:N] = bs
    w2 = np.zeros((K, 2 * CI, 2 * CO), np.float32)
    w2[:, :CI, :CO] = ws
    w2[:, CI:, CO:] = ws
    w2 = round_f32r(w2)
    in_maps = []
    for c in range(NCORES):
        xp = np.zeros((NPAD, 2 * CI), np.float32)
        xp[:N, :CI] = x[2 * c]
        xp[:N, CI:] = x[2 * c + 1]
        in_maps.append({"xpair": xp, "idsw": idsw, "bs": bs_p, "w2": w2})
    return in_maps


def kernel(x, knn_ids, bs, ws):
    from concourse import bass_utils

    x = np.asarray(x, np.float32)
    knn_ids = np.asarray(knn_ids, np.int32)
    bs = np.asarray(bs, np.float32)
    ws = np.asarray(ws, np.float32)

    nc = _get_program()
    in_maps = _pack_inputs(x, knn_ids, bs, ws)
    try:
        res = bass_utils.run_bass_kernel_spmd(
            nc, in_maps, core_ids=list(range(NCORES))
        )
    except Exception:
        # one retry: a crashed previous tenant can leave a core in
        # NRT_EXEC_UNIT_UNRECOVERABLE until the next nrt_init resets it
        res = bass_utils.run_bass_kernel_spmd(
            nc, in_maps, core_ids=list(range(NCORES))
        )

    y = np.empty((B, N, CO), np.float32)
    for c in range(NCORES):
        yt = res.results[c]["yT"]  # [BPC, CO, NPAD]
        for b in range(BPC):
            y[BPC * c + b] = yt[b, :, :N].T
    return y



# revision 7
# speedup vs baseline: 1.2792x; 1.2792x over previous
"""Trainium2 Bass kernel for LocalSLC GNN message passing.

Computation (per batch b):
    y[b,n,o] = sum_{k,i} bs[n,k] * ws[k,i,o] * x[b, knn_ids[n,k], i]

Shapes: B=16, N=10000, K=16, C_IN=C_OUT=64, fp32.

Strategy (8 NeuronCores; batch packed 8-wide in fp16, nodes split 4-way):
  * Host packs x for batch-group g as xq[n, 512] fp16 =
    [x[8g],...,x[8g+7]] rows (1 KiB).  Core c = 4g+q computes nodes
    [2500q, 2500q+2500) for the 8 batches of group g, gathering from
    the FULL node table.
  * Transpose-mode indirect DMAs (512 indices per call — the hw limit
    for transpose gathers) fetch, for one (k, 512-node superblock)
    pair, the 512 neighbor rows straight into the TRANSPOSED layout
    zT[128, k, 4, 512]: partitions = (2 batches x 64 features), free =
    (k, batch-pair c, node).  No PE transposes needed.
  * bs[n,k] scaling: PE rank-1 matmuls (ones[1,128] x bs_row chunks)
    broadcast bs across partitions into PSUM, ACT escapes to fp16 SBUF,
    one in-place DVE multiply per batch-pair scales zT.
  * Per (c, k): fp16 matmul with stationary block-diag W2[k] =
    diag(ws[k], ws[k]) accumulates yT[(2 batches x 64 out), 512] over
    the 16 k's in one PSUM bank; ACT escapes to fp16 and DMAs out.
  * Host transposes yT back and casts to fp32.
"""

import numpy as np

import concourse.bass as bass
import concourse.tile as tile
from concourse import bacc, mybir

B, N, K, CI, CO = 16, 10000, 16, 64, 64
NCORES = 8
GROUPS = 2            # batch groups of 8
QUARTER = N // 4      # 2500 nodes per core before padding
NPC = 2560            # nodes per core (2500 padded to mult of 512)
SB = 512              # nodes per superblock (= indices per gather call)
NSB = NPC // SB       # 5 superblocks
NIDX = SB * K         # 8192 gathered rows per superblock
NPAD = 10240          # table rows (N padded)
EW = 8 * CI           # fp16 words per table row (1 KiB)
CC = EW // 128        # 4 batch-pair column groups


def build_program():
    """Build the per-core Bass program (identical on all 8 cores)."""
    nc = bacc.Bacc("TRN2", target_bir_lowering=False, debug=False)
    f16, f32, i16 = mybir.dt.float16, mybir.dt.float32, mybir.dt.int16

    xq = nc.dram_tensor("xq", [NPAD, EW], f16, kind="ExternalInput").ap()
    # wrapped int16 gather indices, one [128, 32] block per (sb, k) chunk
    idsw = nc.dram_tensor("idsw", [128, NSB * K, SB // 16], i16,
                          kind="ExternalInput").ap()
    bsr = nc.dram_tensor("bsr", [NSB, NIDX], f16, kind="ExternalInput").ap()
    wts = nc.dram_tensor("w2", [K, 2 * CI, 2 * CO], f16,
                         kind="ExternalInput").ap()
    yT = nc.dram_tensor("yT", [CC, 2 * CO, NPC], f16,
                        kind="ExternalOutput").ap()

    with tile.TileContext(nc) as tc:
        with (
            tc.tile_pool(name="const", bufs=1) as const_pool,
            tc.tile_pool(name="meta", bufs=4) as meta_pool,
            tc.tile_pool(name="z", bufs=2) as z_pool,
            tc.tile_pool(name="bsb", bufs=2) as bsb_pool,
            tc.tile_pool(name="ysb", bufs=4) as ysb_pool,
            tc.tile_pool(name="bsp", bufs=3, space="PSUM") as bsp_pool,
            tc.tile_pool(name="yp", bufs=4, space="PSUM") as yp_pool,
        ):
            ones = const_pool.tile([1, 128], f16)
            nc.vector.memset(ones[:], 1.0)
            w2_s = const_pool.tile([128, K, 2 * CO], f16)
            for k in range(K):
                nc.sync.dma_start(out=w2_s[:, k, :], in_=wts[k])
            ids_s = const_pool.tile([128, NSB * K, SB // 16], i16)
            nc.sync.dma_start(out=ids_s[:], in_=idsw[:])

            for s in range(NSB):
                zT = z_pool.tile([128, K, CC, SB], f16, tag="z")
                for k in range(K):
                    nc.gpsimd.dma_gather(
                        out_ap=zT[:, k, :, :],
                        in_ap=xq[:],
                        idxs_ap=ids_s[:, s * K + k, :],
                        num_idxs=SB,
                        num_idxs_reg=SB,
                        elem_size=EW,
                        transpose=True,
                    )

                # broadcast bs rows across partitions: PE rank-1 matmuls
                bsb = bsb_pool.tile([128, K, SB], f16, tag="bsb")
                for k in range(K):
                    bsrow = meta_pool.tile([1, SB], f16, tag="bsr")
                    nc.sync.dma_start(
                        out=bsrow[:],
                        in_=bsr[s:s + 1, k * SB:(k + 1) * SB])
                    bs_ps = bsp_pool.tile([128, SB], f32, tag="bsp")
                    nc.tensor.matmul(
                        bs_ps[:],
                        lhsT=ones[:],
                        rhs=bsrow[:],
                        start=True,
                        stop=True,
                    )
                    nc.scalar.copy(out=bsb[:, k, :], in_=bs_ps[:])

                for c in range(CC):
                    nc.vector.tensor_mul(zT[:, :, c, :], zT[:, :, c, :],
                                         bsb[:])

                for c in range(CC):
                    y_ps = yp_pool.tile([2 * CO, SB], f32, tag="y")
                    for k in range(K):
                        nc.tensor.matmul(
                            y_ps[:],
                            lhsT=w2_s[:, k, :],
                            rhs=zT[:, k, c, :],
                            start=(k == 0),
                            stop=(k == K - 1),
                        )
                    y_sb = ysb_pool.tile([2 * CO, SB], f16, tag="ysb")
                    nc.scalar.copy(out=y_sb[:], in_=y_ps[:])
                    nc.sync.dma_start(out=yT[c, :, s * SB:(s + 1) * SB],
                                      in_=y_sb[:])

    nc.compile()
    return nc


_CACHE = {}


def _get_program():
    if "nc" not in _CACHE:
        _CACHE["nc"] = build_program()
    return _CACHE["nc"]


def _pack_inputs(x, knn_ids, bs, ws):
    """Host-side packing into per-core input maps."""
    # fp16 table per batch group: [NPAD, 512] rows of 8 batches x 64 feats
    xqs = []
    for g in range(GROUPS):
        xq = np.zeros((NPAD, EW), np.float16)
        for b in range(8):
            xq[:N, b * CI:(b + 1) * CI] = x[8 * g + b]
        xqs.append(xq)

    # per-quarter (k, sb)-chunked wrapped indices and k-major bs rows
    idsw_q, bsr_q = [], []
    for q in range(4):
        n0 = q * QUARTER
        ids_flat = np.zeros((NSB, K, SB), np.int32)
        bs_flat = np.zeros((NSB, K, SB), np.float32)
        for s in range(NSB):
            lo = n0 + s * SB
            hi = min(n0 + (s + 1) * SB, n0 + QUARTER)
            nn = hi - lo
            if nn > 0:
                ids_flat[s, :, :nn] = knn_ids[lo:hi].T
                bs_flat[s, :, :nn] = bs[lo:hi].T
        # wrap each 512-idx chunk: w[p, t] = chunk[t*16 + p%16]
        flat = ids_flat.reshape(NSB * K, SB)
        w = flat.reshape(NSB * K, SB // 16, 16).transpose(2, 0, 1)
        w = np.tile(w, (8, 1, 1))  # [128, NSB*K, 32]
        idsw_q.append(np.ascontiguousarray(w.astype(np.int16)))
        bsr_q.append(bs_flat.reshape(NSB, NIDX).astype(np.float16))

    w2 = np.zeros((K, 2 * CI, 2 * CO), np.float16)
    w2[:, :CI, :CO] = ws
    w2[:, CI:, CO:] = ws

    in_maps = []
    for c in range(NCORES):
        g, q = c // 4, c % 4
        in_maps.append({"xq": xqs[g], "idsw": idsw_q[q], "bsr": bsr_q[q],
                        "w2": w2})
    return in_maps


def kernel(x, knn_ids, bs, ws):
    from concourse import bass_utils

    x = np.asarray(x, np.float32)
    knn_ids = np.asarray(knn_ids, np.int32)
    bs = np.asarray(bs, np.float32)
    ws = np.asarray(ws, np.float32)

    nc = _get_program()
    in_maps = _pack_inputs(x, knn_ids, bs, ws)
    try:
        res = bass_utils.run_bass_kernel_spmd(
            nc, in_maps, core_ids=list(range(NCORES))
        )
    except Exception:
        # one retry: a crashed previous tenant can leave a core in
        # NRT_EXEC_UNIT_UNRECOVERABLE until the next nrt_init resets it
        res = bass_utils.run_bass_kernel_spmd(
            nc, in_maps, core_ids=list(range(NCORES))
        )

    y = np.empty((B, N, CO), np.float32)
    for c in range(NCORES):
        g, q = c // 4, c % 4
        n0 = q * QUARTER
        yt = res.results[c]["yT"]  # [CC, 128, NPC] f16
        for cc in range(CC):
            for p in range(2):
                b = 8 * g + 2 * cc + p
                y[b, n0:n0 + QUARTER] = (
                    yt[cc, p * CO:(p + 1) * CO, :QUARTER].T.astype(np.float32))
    return y


# revision 10
# speedup vs baseline: 1.9922x; 1.5574x over previous
"""Trainium2 Bass kernel for LocalSLC GNN message passing.

Computation (per batch b):
    y[b,n,o] = sum_{k,i} bs[n,k] * ws[k,i,o] * x[b, knn_ids[n,k], i]

Shapes: B=16, N=10000, K=16, C_IN=C_OUT=64, fp32.

Strategy (8 NeuronCores; batch packed 8-wide in fp16, nodes split 4-way):
  * Host packs x for batch-group g as xq[n, 512] fp16 =
    [x[8g],...,x[8g+7]] rows (1 KiB).  Core c = 4g+q computes nodes
    [2500q, 2500q+2500) for the 8 batches of group g, gathering from
    the FULL node table.
  * Transpose-mode indirect DMAs (512 indices per call — the hw limit
    for transpose gathers) fetch, for one (k, 512-node superblock)
    pair, the 512 neighbor rows straight into the TRANSPOSED layout
    zT[128, k, 4, 512]: partitions = (2 batches x 64 features), free =
    (k, batch-pair c, node).  No PE transposes needed.
  * bs[n,k] scaling: PE rank-1 matmuls (ones[1,128] x bs_row chunks)
    broadcast bs across partitions into PSUM, ACT escapes to fp16 SBUF,
    one in-place DVE multiply per batch-pair scales zT.
  * Per (c, k): fp16 matmul with stationary block-diag W2[k] =
    diag(ws[k], ws[k]) accumulates yT[(2 batches x 64 out), 512] over
    the 16 k's in one PSUM bank; ACT escapes to fp16 and DMAs out.
  * Host transposes yT back and casts to fp32.
"""

import numpy as np

import concourse.bass as bass
import concourse.tile as tile
from concourse import bacc, mybir

B, N, K, CI, CO = 16, 10000, 16, 64, 64
NCORES = 8
GROUPS = 2            # batch groups of 8
QUARTER = N // 4      # 2500 nodes per core before padding
NPC = 2560            # nodes per core (2500 padded to mult of 512)
SB = 512              # nodes per superblock (= indices per gather call)
NSB = NPC // SB       # 5 superblocks
NIDX = SB * K         # 8192 gathered rows per superblock
NPAD = 10240          # table rows (N padded)
EW = 8 * CI           # fp16 words per table row (1 KiB)
CC = EW // 128        # 4 batch-pair column groups


def build_program():
    """Build the per-core Bass program (identical on all 8 cores)."""
    nc = bacc.Bacc("TRN2", target_bir_lowering=False, debug=False)
    f16, f32, i16 = mybir.dt.float16, mybir.dt.float32, mybir.dt.int16

    xq = nc.dram_tensor("xq", [NPAD, EW], f16, kind="ExternalInput").ap()
    # wrapped int16 gather indices, one [128, 32] block per (sb, k) chunk
    idsw = nc.dram_tensor("idsw", [128, NSB * K, SB // 16], i16,
                          kind="ExternalInput").ap()
    bsr = nc.dram_tensor("bsr", [NSB, NIDX], f16, kind="ExternalInput").ap()
    wts = nc.dram_tensor("w2", [K, 2 * CI, 2 * CO], f16,
                         kind="ExternalInput").ap()
    yT = nc.dram_tensor("yT", [CC, 2 * CO, NPC], f16,
                        kind="ExternalOutput").ap()

    with tile.TileContext(nc) as tc:
        with (
            tc.tile_pool(name="const", bufs=1) as const_pool,
            tc.tile_pool(name="meta", bufs=4) as meta_pool,
            tc.tile_pool(name="z", bufs=2) as z_pool,
            tc.tile_pool(name="bsb", bufs=2) as bsb_pool,
            tc.tile_pool(name="ysb", bufs=4) as ysb_pool,
            tc.tile_pool(name="bsp", bufs=3, space="PSUM") as bsp_pool,
            tc.tile_pool(name="yp", bufs=1, space="PSUM") as yp_pool,
        ):
            ones = const_pool.tile([1, 128], f16)
            nc.vector.memset(ones[:], 1.0)
            w2_s = const_pool.tile([128, K, 2 * CO], f16)
            for k in range(K):
                nc.sync.dma_start(out=w2_s[:, k, :], in_=wts[k])
            ids_s = const_pool.tile([128, NSB * K, SB // 16], i16)
            nc.sync.dma_start(out=ids_s[:], in_=idsw[:])

            for s in range(NSB):
                zT = z_pool.tile([128, K, CC, SB], f16, tag="z")
                bsb = bsb_pool.tile([128, K, SB], f16, tag="bsb")
                y_ps = [yp_pool.tile([2 * CO, SB], f32, tag=f"y{c}",
                                     name=f"yps{c}")
                        for c in range(CC)]
                bsrows = []
                for half in range(2):
                    bsrow = meta_pool.tile([1, K // 2 * SB], f16, tag="bsr")
                    nc.sync.dma_start(
                        out=bsrow[:],
                        in_=bsr[s:s + 1,
                                half * (K // 2) * SB:(half + 1) * (K // 2) * SB])
                    bsrows.append(bsrow)
                for k in range(K):
                    nc.gpsimd.dma_gather(
                        out_ap=zT[:, k, :, :],
                        in_ap=xq[:],
                        idxs_ap=ids_s[:, s * K + k, :],
                        num_idxs=SB,
                        num_idxs_reg=SB,
                        elem_size=EW,
                        transpose=True,
                    )
                    # broadcast bs row across partitions: PE rank-1 matmul
                    kk = k % (K // 2)
                    bs_ps = bsp_pool.tile([128, SB], f32, tag="bsp")
                    nc.tensor.matmul(
                        bs_ps[:],
                        lhsT=ones[:],
                        rhs=bsrows[k // (K // 2)][:1, kk * SB:(kk + 1) * SB],
                        start=True,
                        stop=True,
                    )
                    nc.scalar.copy(out=bsb[:, k, :], in_=bs_ps[:])
                    nc.vector.tensor_mul(
                        zT[:, k, :, :], zT[:, k, :, :],
                        bsb[:, k, :].unsqueeze(1).broadcast_to([128, CC, SB]))
                    for c in range(CC):
                        nc.tensor.matmul(
                            y_ps[c][:],
                            lhsT=w2_s[:, k, :],
                            rhs=zT[:, k, c, :],
                            start=(k == 0),
                            stop=(k == K - 1),
                        )
                for c in range(CC):
                    y_sb = ysb_pool.tile([2 * CO, SB], f16, tag="ysb")
                    nc.scalar.copy(out=y_sb[:], in_=y_ps[c][:])
                    nc.sync.dma_start(out=yT[c, :, s * SB:(s + 1) * SB],
                                      in_=y_sb[:])

    nc.compile()
    return nc


_CACHE = {}


def _get_program():
    if "nc" not in _CACHE:
        _CACHE["nc"] = build_program()
    return _CACHE["nc"]


def _pack_inputs(x, knn_ids, bs, ws):
    """Host-side packing into per-core input maps."""
    # fp16 table per batch group: [NPAD, 512] rows of 8 batches x 64 feats
    xqs = []
    for g in range(GROUPS):
        xq = np.zeros((NPAD, EW), np.float16)
        for b in range(8):
            xq[:N, b * CI:(b + 1) * CI] = x[8 * g + b]
        xqs.append(xq)

    # per-quarter (k, sb)-chunked wrapped indices and k-major bs rows
    idsw_q, bsr_q = [], []
    for q in range(4):
        n0 = q * QUARTER
        ids_flat = np.zeros((NSB, K, SB), np.int32)
        bs_flat = np.zeros((NSB, K, SB), np.float32)
        for s in range(NSB):
            lo = n0 + s * SB
            hi = min(n0 + (s + 1) * SB, n0 + QUARTER)
            nn = hi - lo
            if nn > 0:
                ids_flat[s, :, :nn] = knn_ids[lo:hi].T
                bs_flat[s, :, :nn] = bs[lo:hi].T
        # wrap each 512-idx chunk: w[p, t] = chunk[t*16 + p%16]
        flat = ids_flat.reshape(NSB * K, SB)
        w = flat.reshape(NSB * K, SB // 16, 16).transpose(2, 0, 1)
        w = np.tile(w, (8, 1, 1))  # [128, NSB*K, 32]
        idsw_q.append(np.ascontiguousarray(w.astype(np.int16)))
        bsr_q.append(bs_flat.reshape(NSB, NIDX).astype(np.float16))

    w2 = np.zeros((K, 2 * CI, 2 * CO), np.float16)
    w2[:, :CI, :CO] = ws
    w2[:, CI:, CO:] = ws

    in_maps = []
    for c in range(NCORES):
        g, q = c // 4, c % 4
        in_maps.append({"xq": xqs[g], "idsw": idsw_q[q], "bsr": bsr_q[q],
                        "w2": w2})
    return in_maps


def kernel(x, knn_ids, bs, ws):
    from concourse import bass_utils

    x = np.asarray(x, np.float32)
    knn_ids = np.asarray(knn_ids, np.int32)
    bs = np.asarray(bs, np.float32)
    ws = np.asarray(ws, np.float32)

    nc = _get_program()
    in_maps = _pack_inputs(x, knn_ids, bs, ws)
    try:
        res = bass_utils.run_bass_kernel_spmd(
            nc, in_maps, core_ids=list(range(NCORES))
        )
    except Exception:
        # one retry: a crashed previous tenant can leave a core in
        # NRT_EXEC_UNIT_UNRECOVERABLE until the next nrt_init resets it
        res = bass_utils.run_bass_kernel_spmd(
            nc, in_maps, core_ids=list(range(NCORES))
        )

    y = np.empty((B, N, CO), np.float32)
    for c in range(NCORES):
        g, q = c // 4, c % 4
        n0 = q * QUARTER
        yt = res.results[c]["yT"]  # [CC, 128, NPC] f16
        for cc in range(CC):
            for p in range(2):
                b = 8 * g + 2 * cc + p
                y[b, n0:n0 + QUARTER] = (
                    yt[cc, p * CO:(p + 1) * CO, :QUARTER].T.astype(np.float32))
    return y


# revision 11
# speedup vs baseline: 2.1396x; 1.0740x over previous
"""Trainium2 Bass kernel for LocalSLC GNN message passing.

Computation (per batch b):
    y[b,n,o] = sum_{k,i} bs[n,k] * ws[k,i,o] * x[b, knn_ids[n,k], i]

Shapes: B=16, N=10000, K=16, C_IN=C_OUT=64, fp32.

Strategy (8 NeuronCores; batch packed 8-wide in fp16, nodes split 4-way):
  * Host packs x for batch-group g as xq[n, 512] fp16 =
    [x[8g],...,x[8g+7]] rows (1 KiB).  Core c = 4g+q computes nodes
    [2500q, 2500q+2500) for the 8 batches of group g, gathering from
    the FULL node table.
  * Transpose-mode indirect DMAs (512 indices per call — the hw limit
    for transpose gathers) fetch, for one (k, 512-node superblock)
    pair, the 512 neighbor rows straight into the TRANSPOSED layout
    zT[128, k, 4, 512]: partitions = (2 batches x 64 features), free =
    (k, batch-pair c, node).  No PE transposes needed.
  * bs[n,k] scaling: PE rank-1 matmuls (ones[1,128] x bs_row chunks)
    broadcast bs across partitions into PSUM, ACT escapes to fp16 SBUF,
    one in-place DVE multiply per batch-pair scales zT.
  * Per (c, k): fp16 matmul with stationary block-diag W2[k] =
    diag(ws[k], ws[k]) accumulates yT[(2 batches x 64 out), 512] over
    the 16 k's in one PSUM bank; ACT escapes to fp16 and DMAs out.
  * Host transposes yT back and casts to fp32.
"""

import numpy as np

import concourse.bass as bass
import concourse.tile as tile
from concourse import bacc, mybir

B, N, K, CI, CO = 16, 10000, 16, 64, 64
NCORES = 8
GROUPS = 2            # batch groups of 8
QUARTER = N // 4      # 2500 nodes per core before padding
NPC = 2560            # nodes per core (2500 padded to mult of 512)
SB = 512              # nodes per superblock (= indices per gather call)
NSB = NPC // SB       # 5 superblocks
NIDX = SB * K         # 8192 gathered rows per superblock
NPAD = 10240          # table rows (N padded)
EW = 8 * CI           # fp16 words per table row (1 KiB)
CC = EW // 128        # 4 batch-pair column groups


def build_program():
    """Build the per-core Bass program (identical on all 8 cores)."""
    nc = bacc.Bacc("TRN2", target_bir_lowering=False, debug=False)
    f16, f32, i16 = mybir.dt.float16, mybir.dt.float32, mybir.dt.int16

    xq = nc.dram_tensor("xq", [NPAD, EW], f16, kind="ExternalInput").ap()
    # wrapped int16 gather indices, one [128, 32] block per (sb, k) chunk
    idsw = nc.dram_tensor("idsw", [128, NSB * K, SB // 16], i16,
                          kind="ExternalInput").ap()
    bsr = nc.dram_tensor("bsr", [NSB, NIDX], f16, kind="ExternalInput").ap()
    wts = nc.dram_tensor("w2", [K, 2 * CI, 2 * CO], f16,
                         kind="ExternalInput").ap()
    yT = nc.dram_tensor("yT", [CC, 2 * CO, NPC], f16,
                        kind="ExternalOutput").ap()

    with tile.TileContext(nc) as tc:
        with (
            tc.tile_pool(name="const", bufs=1) as const_pool,
            tc.tile_pool(name="meta", bufs=4) as meta_pool,
            tc.tile_pool(name="z", bufs=2) as z_pool,
            tc.tile_pool(name="bsb", bufs=2) as bsb_pool,
            tc.tile_pool(name="ysb", bufs=4) as ysb_pool,
            tc.tile_pool(name="bsp", bufs=3, space="PSUM") as bsp_pool,
            tc.tile_pool(name="yp", bufs=1, space="PSUM") as yp_pool,
        ):
            ids_s = const_pool.tile([128, NSB * K, SB // 16], i16)
            nc.sync.dma_start(out=ids_s[:], in_=idsw[:])
            ones = const_pool.tile([1, 128], f16)
            nc.vector.memset(ones[:], 1.0)
            w2_s = const_pool.tile([128, K, 2 * CO], f16)
            nc.sync.dma_start(out=w2_s[:],
                              in_=wts.rearrange("k i o -> i k o"))

            for s in range(NSB):
                zT = z_pool.tile([128, K, CC, SB], f16, tag="z")
                bsb = bsb_pool.tile([128, K, SB], f16, tag="bsb")
                y_ps = [yp_pool.tile([2 * CO, SB], f32, tag=f"y{c}",
                                     name=f"yps{c}")
                        for c in range(CC)]
                bsrows = []
                for half in range(2):
                    bsrow = meta_pool.tile([1, K // 2 * SB], f16, tag="bsr")
                    nc.sync.dma_start(
                        out=bsrow[:],
                        in_=bsr[s:s + 1,
                                half * (K // 2) * SB:(half + 1) * (K // 2) * SB])
                    bsrows.append(bsrow)
                for k in range(K):
                    nc.gpsimd.dma_gather(
                        out_ap=zT[:, k, :, :],
                        in_ap=xq[:],
                        idxs_ap=ids_s[:, s * K + k, :],
                        num_idxs=SB,
                        num_idxs_reg=SB,
                        elem_size=EW,
                        transpose=True,
                    )
                    # broadcast bs row across partitions: PE rank-1 matmul
                    kk = k % (K // 2)
                    bs_ps = bsp_pool.tile([128, SB], f32, tag="bsp")
                    nc.tensor.matmul(
                        bs_ps[:],
                        lhsT=ones[:],
                        rhs=bsrows[k // (K // 2)][:1, kk * SB:(kk + 1) * SB],
                        start=True,
                        stop=True,
                    )
                    nc.scalar.copy(out=bsb[:, k, :], in_=bs_ps[:])
                    nc.vector.tensor_mul(
                        zT[:, k, :, :], zT[:, k, :, :],
                        bsb[:, k, :].unsqueeze(1).broadcast_to([128, CC, SB]))
                    for c in range(CC):
                        nc.tensor.matmul(
                            y_ps[c][:],
                            lhsT=w2_s[:, k, :],
                            rhs=zT[:, k, c, :],
                            start=(k == 0),
                            stop=(k == K - 1),
                        )
                for c in range(CC):
                    y_sb = ysb_pool.tile([2 * CO, SB], f16, tag="ysb")
                    nc.scalar.copy(out=y_sb[:], in_=y_ps[c][:])
                    nc.sync.dma_start(out=yT[c, :, s * SB:(s + 1) * SB],
                                      in_=y_sb[:])

    nc.compile()
    return nc


_CACHE = {}


def _get_program():
    if "nc" not in _CACHE:
        _CACHE["nc"] = build_program()
    return _CACHE["nc"]


def _pack_inputs(x, knn_ids, bs, ws):
    """Host-side packing into per-core input maps."""
    # fp16 table per batch group: [NPAD, 512] rows of 8 batches x 64 feats
    xqs = []
    for g in range(GROUPS):
        xq = np.zeros((NPAD, EW), np.float16)
        for b in range(8):
            xq[:N, b * CI:(b + 1) * CI] = x[8 * g + b]
        xqs.append(xq)

    # per-quarter (k, sb)-chunked wrapped indices and k-major bs rows
    idsw_q, bsr_q = [], []
    for q in range(4):
        n0 = q * QUARTER
        ids_flat = np.zeros((NSB, K, SB), np.int32)
        bs_flat = np.zeros((NSB, K, SB), np.float32)
        for s in range(NSB):
            lo = n0 + s * SB
            hi = min(n0 + (s + 1) * SB, n0 + QUARTER)
            nn = hi - lo
            if nn > 0:
                ids_flat[s, :, :nn] = knn_ids[lo:hi].T
                bs_flat[s, :, :nn] = bs[lo:hi].T
        # wrap each 512-idx chunk: w[p, t] = chunk[t*16 + p%16]
        flat = ids_flat.reshape(NSB * K, SB)
        w = flat.reshape(NSB * K, SB // 16, 16).transpose(2, 0, 1)
        w = np.tile(w, (8, 1, 1))  # [128, NSB*K, 32]
        idsw_q.append(np.ascontiguousarray(w.astype(np.int16)))
        bsr_q.append(bs_flat.reshape(NSB, NIDX).astype(np.float16))

    w2 = np.zeros((K, 2 * CI, 2 * CO), np.float16)
    w2[:, :CI, :CO] = ws
    w2[:, CI:, CO:] = ws

    in_maps = []
    for c in range(NCORES):
        g, q = c // 4, c % 4
        in_maps.append({"xq": xqs[g], "idsw": idsw_q[q], "bsr": bsr_q[q],
                        "w2": w2})
    return in_maps


def kernel(x, knn_ids, bs, ws):
    from concourse import bass_utils

    x = np.asarray(x, np.float32)
    knn_ids = np.asarray(knn_ids, np.int32)
    bs = np.asarray(bs, np.float32)
    ws = np.asarray(ws, np.float32)

    nc = _get_program()
    in_maps = _pack_inputs(x, knn_ids, bs, ws)
    try:
        res = bass_utils.run_bass_kernel_spmd(
            nc, in_maps, core_ids=list(range(NCORES))
        )
    except Exception:
        # one retry: a crashed previous tenant can leave a core in
        # NRT_EXEC_UNIT_UNRECOVERABLE until the next nrt_init resets it
        res = bass_utils.run_bass_kernel_spmd(
            nc, in_maps, core_ids=list(range(NCORES))
        )

    y = np.empty((B, N, CO), np.float32)
    for c in range(NCORES):
        g, q = c // 4, c % 4
        n0 = q * QUARTER
        yt = res.results[c]["yT"]  # [CC, 128, NPC] f16
        for cc in range(CC):
            for p in range(2):
                b = 8 * g + 2 * cc + p
                y[b, n0:n0 + QUARTER] = (
                    yt[cc, p * CO:(p + 1) * CO, :QUARTER].T.astype(np.float32))
    return y
